# revision 9
# baseline (speedup 1.0000x reference)
"""Trainium2 Bass kernel for fused cross-entropy + focal-scaled sum loss.

Computes, for logits X [N, 128] (f32) and integer targets t [N]:
    ce_i   = logsumexp(X_i) - X_i[t_i]
    ce     = sum_i ce_i
    loss   = (1 - exp(-ce))**2 * ce

Strategy (8 NeuronCores, data parallel over N):
  - Each core processes R = N/8 consecutive rows, tiled as [128 partitions,
    K rows/partition, 128 classes] (128*K rows per tile).
  - ACT: E = exp(X) on the full tile.
  - DVE: segmented row-sum of E -> S (one wide tensor_reduce per tile).
  - Gather sum_rows X[i, t_i]: one custom fused DVE op per tile:
    body (Idx == t_enc) * X with add-accumulate, where t_enc = 128*k + t
    is precomputed on host and broadcast along the class dim via a
    stride-0 AP.  (v1 fallback: K scalar_tensor_tensor slices.)
  - End: one batched Ln pass over all S, two wide reduces, DMA out the
    per-partition partial sums [128, 2]; host sums 8x128x2 partials and
    applies the focal scaling exactly as the reference does in f32.
"""

import os

import numpy as np

N_CORES = 8
N_FULL = 2097152
C = 128
R_CORE = N_FULL // N_CORES  # 262144 rows per core

K_DEFAULT = 16  # rows per partition per tile
VARIANT_DEFAULT = os.environ.get("CE_VARIANT", "v2")

_CE_GATHER_OP = None


def _register_ce_gather():
    """Register the fused gather op: out = (Idx == in1) * in0, accum += out.

    One wide DVE instruction per tile replaces K scalar_tensor_tensor
    slices.  in1 is the host-precomputed flattened target position
    (128*k + t), broadcast along the class dim with a stride-0 AP.
    """
    global _CE_GATHER_OP
    if _CE_GATHER_OP is not None:
        return _CE_GATHER_OP

    from operator import add

    import concourse.dve_ops as dve_ops
    from concourse.dve_spec import Spec, Src0, Src1, Idx, eq, lower, _has_src1
    from concourse.dve_uop import DveOpSpec

    name = "CE_GATHER_ANT"
    for op in dve_ops.OPS:
        if op.name == name:
            _CE_GATHER_OP = op
            return op

    def _ref(in0, in1, s0, s1, imm2):
        p = in0.shape[0]
        x = np.asarray(in0, np.float32).reshape(p, -1)
        tt = np.asarray(in1, np.float32).reshape(p, -1)
        idx = np.broadcast_to(
            np.arange(x.shape[1], dtype=np.float32), x.shape
        )
        b = np.where(idx == tt, x, 0.0).astype(np.float32)
        return b.reshape(in0.shape), b.sum(axis=-1, keepdims=True).astype(
            np.float32
        )

    spec = Spec(body=eq(Idx, Src1) * Src0, accum=add, reference=_ref)
    row = dve_ops._CUSTOM_DVE_ROW_BASE + len(dve_ops.OPS)
    shas = {}
    for ver in ("v3", "v4"):
        tmp = DveOpSpec(
            name=name, opcode=row, uops=lower(spec, ver=ver),
            rd1_en=_has_src1(spec),
        )
        shas[ver] = tmp.sha(ver)
    op = dve_ops.DveOp(name, spec, subdim=False, uops_sha=shas)
    dve_ops.OPS.append(op)
    dve_ops._SUB_OPCODE_FOR_NAME[name] = row
    dve_ops.CUSTOM_DVE_SPECS[name] = spec
    _CE_GATHER_OP = op
    return op


def build_program(
    R, K=K_DEFAULT, variant=VARIANT_DEFAULT, n_devices=N_CORES, repeat=1
):
    """Build the SPMD Bass program for one core processing R rows.

    repeat > 1 re-runs the whole tile loop over the same input (for
    benchmarking: device work scales by `repeat`, host/RPC cost doesn't).
    """
    from contextlib import ExitStack

    import concourse.bacc as bacc
    import concourse.mybir as mybir
    import concourse.tile as tile

    F = K * C
    rows_per_tile = 128 * K
    assert R % rows_per_tile == 0
    T = R // rows_per_tile

    gather_op = _register_ce_gather() if variant in ("v2", "v3") else None

    dt = mybir.dt
    nc = bacc.Bacc(
        "TRN2", target_bir_lowering=False, debug=False, num_devices=n_devices
    )

    x_d = nc.dram_tensor("x", [R, C], dt.float32, kind="ExternalInput")
    tgt_d = nc.dram_tensor("tgt", [128, T * K], dt.float32, kind="ExternalInput")
    iota_d = nc.dram_tensor("iota", [128, C], dt.float32, kind="ExternalInput")
    out_d = nc.dram_tensor("out", [128, 2], dt.float32, kind="ExternalOutput")

    x_view = x_d.ap().rearrange("(t p k) c -> t p k c", p=128, k=K)

    with tile.TileContext(nc) as tc, ExitStack() as ctx:
        const_pool = ctx.enter_context(tc.tile_pool(name="const", bufs=1))
        xpool = ctx.enter_context(tc.tile_pool(name="x", bufs=3))
        epool = ctx.enter_context(tc.tile_pool(name="e", bufs=2))
        tpool = ctx.enter_context(tc.tile_pool(name="tree", bufs=2))

        s_dt = dt.bfloat16 if variant == "v3" else dt.float32
        iota_sb = const_pool.tile([128, C], dt.float32, tag="iota")
        t_sb = const_pool.tile([128, T * K], dt.float32, tag="tgt")
        s_all = const_pool.tile([128, T * K], s_dt, tag="s_all")
        ncols = T if variant in ("v2", "v3") else T * K
        g_all = const_pool.tile([128, ncols], dt.float32, tag="g_all")
        l_all = const_pool.tile([128, T * K], dt.float32, tag="l_all")
        scratch = const_pool.tile([128, F], dt.float32, tag="scratch")
        red = const_pool.tile([128, 2], dt.float32, tag="red")

        nc.sync.dma_start(iota_sb[:], iota_d.ap())
        nc.sync.dma_start(t_sb[:], tgt_d.ap())

        for i in range(T * repeat):
            i = i % T
            xt = xpool.tile([128, F], dt.float32, tag="xt")
            nc.sync.dma_start(
                xt[:].rearrange("p (k c) -> p k c", c=C), x_view[i]
            )
            et = epool.tile(
                [128, F], dt.bfloat16 if variant == "v3" else dt.float32,
                tag="et",
            )
            nc.scalar.activation(et[:], xt[:], mybir.ActivationFunctionType.Exp)
            if variant == "v3":
                # pairwise bf16 add tree (tensor_tensor runs 2x on bf16;
                # tensor_reduce has no fast mode)
                cur = et[:].rearrange("p (k c) -> p k c", c=C)
                w = C
                while w > 2:
                    nxt = tpool.tile(
                        [128, K * (w // 2)], dt.bfloat16, tag=f"tree{w}"
                    )
                    nxt3 = nxt[:].rearrange("p (k c) -> p k c", c=w // 2)
                    nc.vector.tensor_tensor(
                        nxt3,
                        cur[:, :, 0 : w // 2],
                        cur[:, :, w // 2 : w],
                        mybir.AluOpType.add,
                    )
                    cur = nxt3
                    w //= 2
                nc.vector.tensor_tensor(
                    s_all[:, i * K : (i + 1) * K],
                    cur[:, :, 0],
                    cur[:, :, 1],
                    mybir.AluOpType.add,
                )
            else:
                nc.vector.tensor_reduce(
                    s_all[:, i * K : (i + 1) * K],
                    et[:].rearrange("p (k c) -> p k c", c=C),
                    axis=mybir.AxisListType.X,
                    op=mybir.AluOpType.add,
                )
            if variant in ("v2", "v3"):
                t_b = (
                    t_sb[:, i * K : (i + 1) * K]
                    .broadcast_to([128, K, C])
                )
                nc.vector._custom_dve(
                    gather_op,
                    out=scratch[:].rearrange("p (k c) -> p k c", c=C),
                    in0=xt[:].rearrange("p (k c) -> p k c", c=C),
                    in1=t_b,
                    accum_out=g_all[:, i : i + 1],
                )
            else:
                for k in range(K):
                    col = i * K + k
                    nc.vector.scalar_tensor_tensor(
                        scratch[:, 0:C],
                        iota_sb[:],
                        t_sb[:, col : col + 1],
                        xt[:, k * C : (k + 1) * C],
                        mybir.AluOpType.is_equal,
                        mybir.AluOpType.mult,
                        accum_out=g_all[:, col : col + 1],
                    )

        nc.scalar.activation(l_all[:], s_all[:], mybir.ActivationFunctionType.Ln)
        nc.vector.tensor_reduce(
            red[:, 0:1], l_all[:], axis=mybir.AxisListType.X, op=mybir.AluOpType.add
        )
        nc.vector.tensor_reduce(
            red[:, 1:2], g_all[:], axis=mybir.AxisListType.X, op=mybir.AluOpType.add
        )
        nc.sync.dma_start(out_d.ap(), red[:])

    nc.compile()
    return nc


def prep_core_inputs(x_shard, t_shard, K=K_DEFAULT, variant=VARIANT_DEFAULT):
    """Host-side input prep for one core's shard (reshape/cast only)."""
    R = x_shard.shape[0]
    rows_per_tile = 128 * K
    T = R // rows_per_tile
    # tgt layout: tgt[p, i*K + k] = enc(t[i*rows_per_tile + p*K + k])
    tgt = t_shard.astype(np.float32).reshape(T, 128, K)
    if variant in ("v2", "v3"):
        # encode flattened position within the [K, C] free block
        tgt = tgt + (np.arange(K, dtype=np.float32) * C)[None, None, :]
    tgt = np.ascontiguousarray(tgt.transpose(1, 0, 2)).reshape(128, T * K)
    iota = np.broadcast_to(np.arange(C, dtype=np.float32)[None, :], (128, C)).copy()
    return {"x": np.ascontiguousarray(x_shard), "tgt": tgt, "iota": iota}


def finalize(per_core_results):
    """Combine per-core [128, 2] partials into the final focal loss (f32)."""
    lse_sum = 0.0
    g_sum = 0.0
    for r in per_core_results:
        red = r["out"]
        lse_sum += float(np.sum(red[:, 0], dtype=np.float64))
        g_sum += float(np.sum(red[:, 1], dtype=np.float64))
    ce = np.float32(lse_sum - g_sum)
    pt = np.exp(-ce).astype(np.float32)
    loss = (np.float32(1.0) - pt) ** 2 * ce
    return np.asarray(loss, dtype=np.float32)


_PROGRAM_CACHE = {}


def _get_program():
    key = (R_CORE, K_DEFAULT, VARIANT_DEFAULT)
    if key not in _PROGRAM_CACHE:
        _PROGRAM_CACHE[key] = build_program(R_CORE)
    return _PROGRAM_CACHE[key]


def kernel(outputs, targets):
    """outputs: [N, 128] f32 logits; targets: [N] int. Returns f32 scalar."""
    from concourse.bass_utils import run_bass_kernel_spmd

    outputs = np.asarray(outputs)
    targets = np.asarray(targets)
    assert outputs.shape == (N_FULL, C), outputs.shape

    nc = _get_program()

    in_maps = []
    for c in range(N_CORES):
        sl = slice(c * R_CORE, (c + 1) * R_CORE)
        in_maps.append(prep_core_inputs(outputs[sl], targets[sl]))

    res = run_bass_kernel_spmd(nc, in_maps, list(range(N_CORES)))
    return finalize(res.results)
